# revision 1
# baseline (speedup 1.0000x reference)
"""DigitCaps dynamic-routing kernel for 8 Trainium2 NeuronCores.

Strategy: shard the num_route_nodes axis (R=2048 -> 256 per core).
  - Phase 1: u_hat production. Per route r: u[b, (c,m)] = xT_r[k,b].T @ w_r[k,(c,m)]
    on the tensor engine (fp32). u staged in device DRAM; the first routing
    iteration (c uniform = 1/CAPS) is fused in as a running sum over routes.
  - Phase 2: each remaining routing iteration is ONE streaming pass over u:
    per r-tile: dot = sum_m u*v  ->  b_logits += dot -> softmax over caps
    (tile-local) -> s_partial += sum_r c*u.  s is AllReduced across cores
    (contraction over r spans cores), squash computed redundantly per core.

Inputs are sharded host-side: x -> xT[k, r_loc, b] slices, w -> w[r_loc, k, c, m]
slices (transpose is layout prep for DMA/matmul efficiency; all FLOPs on device).
"""

import os
import sys

if "/opt/trn_rl_repo" not in sys.path:
    sys.path.insert(0, "/opt/trn_rl_repo")

import numpy as np

B, R, K, C, M = 128, 2048, 64, 32, 32
CM = C * M
N_CORES = 8
R_LOC = R // N_CORES
RT1 = int(os.environ.get("DC_RT1", "8"))   # routes per tile, u-production
RT2 = int(os.environ.get("DC_RT2", "8"))  # routes per tile, routing passes
S1_ON_PE = os.environ.get("DC_S1PE", "1") == "1"
SIM_MODE = os.environ.get("DC_SIM", "0") == "1"   # 1-core, collective->copy
V_EXP = os.environ.get("DC_VEXP", "1") == "1"     # materialize v expanded
C_EXP = os.environ.get("DC_CEXP", "0") == "1"     # materialize coef expanded (ACT)
Q_SWAP = os.environ.get("DC_QSWAP", "0") == "1"   # coef as in0 in the q product

PROD_ENGINE = os.environ.get("DC_PROD", "vector")   # "vector" | "gpsimd"
U_DT = os.environ.get("DC_U_DT", "float16")         # staged-u dtype
MM_DT = os.environ.get("DC_MM", "float32r")         # matmul input dtype

_compiled = {}
LAST_RESULT = None          # BassKernelResults of the most recent run (for test.py)


def _view(ap, dims):
    """Free-dim view of an AP: keep its partition dim, replace free dims by
    [step, count] pairs (element steps). step 0 = broadcast."""
    import concourse.bass as bass

    return bass.AP(
        tensor=ap.tensor,
        offset=ap.offset,
        ap=[list(ap.ap[0])] + [[s, c] for s, c in dims],
    )


def _ap(ap, dims):
    """Fully custom AP (all dims given) at the base offset of `ap`."""
    import concourse.bass as bass

    return bass.AP(
        tensor=ap.tensor,
        offset=ap.offset,
        ap=[[s, c] for s, c in dims],
    )


def _squash(nc, pool, s_ap, v_ap):
    """v = s * |s|^2 / ((1 + |s|^2) (sqrt(|s|^2) + 1e-8)), norm over m."""
    import concourse.mybir as mybir

    f32 = mybir.dt.float32
    op = mybir.AluOpType
    sq_full = pool.tile([B, CM], f32, tag="sq_full")
    nc.vector.tensor_tensor(sq_full[:], s_ap, s_ap, op=op.mult)
    sq = pool.tile([B, C], f32, tag="sq")
    nc.vector.tensor_reduce(
        sq[:], _view(sq_full[:], [(1, C), (C, M)]), axis=mybir.AxisListType.X,
        op=op.add)
    rt = pool.tile([B, C], f32, tag="rt")
    nc.scalar.activation(rt[:], sq[:], mybir.ActivationFunctionType.Sqrt)
    nc.vector.tensor_scalar(rt[:], rt[:], 1e-8, None, op0=op.add)
    den = pool.tile([B, C], f32, tag="den")
    nc.vector.tensor_scalar(den[:], sq[:], 1.0, None, op0=op.add)
    nc.vector.tensor_tensor(den[:], den[:], rt[:], op=op.mult)
    fi = pool.tile([B, C], f32, tag="fi")
    nc.vector.reciprocal(fi[:], den[:])
    nc.vector.tensor_tensor(fi[:], fi[:], sq[:], op=op.mult)
    # v = s * f (f broadcast over m)
    nc.vector.tensor_tensor(
        v_ap,
        _view(s_ap, [(C, M), (1, C)]),
        _view(fi[:], [(0, M), (1, C)]),
        op=op.mult,
    )


def _build(n_iters, repeat=1):
    import concourse.mybir as mybir
    import concourse.tile as tile
    from concourse import bacc

    f32 = mybir.dt.float32
    u_dt = getattr(mybir.dt, U_DT)
    mm_dt = getattr(mybir.dt, MM_DT)
    op = mybir.AluOpType
    AX = mybir.AxisListType

    nc = bacc.Bacc("TRN2", target_bir_lowering=False, debug=False,
                   num_devices=1 if SIM_MODE else N_CORES)
    xT = nc.dram_tensor("xT", [R_LOC // 2, 2, K, B], mm_dt,
                        kind="ExternalInput").ap()
    wT = nc.dram_tensor("wT", [R_LOC // 2, 2, K, CM], mm_dt,
                        kind="ExternalInput").ap()
    out = nc.dram_tensor("out", [B, CM], f32, kind="ExternalOutput").ap()

    if PROD_ENGINE == "split":
        prod_p, prod_q = nc.vector, nc.gpsimd
    else:
        prod_p = prod_q = {"gpsimd": nc.gpsimd, "vector": nc.vector}[PROD_ENGINE]

    with tile.TileContext(nc) as tc:
        with (
            tc.tile_pool(name="sm", bufs=2) as sm,       # small temps
            tc.tile_pool(name="persist", bufs=1) as persist,
            tc.tile_pool(name="dram", bufs=1, space="DRAM") as dram,
            tc.tile_pool(name="drbounce", bufs=min(2 * n_iters * repeat, 8),
                         space="DRAM") as drb,
        ):
            u_dram = dram.tile([B, R_LOC * CM], u_dt)
            b_log = persist.tile([B, R_LOC * C], f32)   # logits, layout (r, c)
            v_sb = persist.tile([B, CM], f32)           # current v (fp32)
            if u_dt != f32:
                v_u = persist.tile([B, CM], u_dt, tag="v_u")
            else:
                v_u = v_sb

            def allreduce_squash(s_acc_tile, scale):
                bin_ = drb.tile([B, CM], f32, tag="bin")
                bout = drb.tile([B, CM], f32, tag="bout")
                nc.sync.dma_start(bin_[:], s_acc_tile[:])
                if SIM_MODE:
                    nc.sync.dma_start(bout[:], bin_[:])
                else:
                    nc.gpsimd.collective_compute(
                        "AllReduce", op.add,
                        replica_groups=[list(range(N_CORES))],
                        ins=[bin_.opt()], outs=[bout.opt()],
                    )
                s_sb = sm.tile([B, CM], f32, tag="s_sb")
                nc.sync.dma_start(s_sb[:], bout[:])
                if scale != 1.0:
                    nc.vector.tensor_scalar(s_sb[:], s_sb[:], scale, None,
                                            op0=op.mult)
                _squash(nc, sm, s_sb[:], v_sb[:])
                if not V_EXP and v_u is not v_sb:
                    nc.vector.tensor_copy(v_u[:], v_sb[:])

            def emit_phase1_packed():
                """u production with route-pairs packed on 128 partitions;
                iteration-1 s accumulated on the PE in a dedicated PSUM pair
                via K=128 packed matmuls (u_r0 + u_r1 per pair)."""
                s_acc = sm.tile([B, CM], f32, tag="s_acc")
                n_tiles = R_LOC // RT1
                half = RT1 // 2
                with (
                    tc.tile_pool(name="xp", bufs=3) as xp,
                    tc.tile_pool(name="wp", bufs=3) as wp,
                    tc.tile_pool(name="up1", bufs=3) as up1,
                    tc.tile_pool(name="pp", bufs=3, space="PSUM") as pp,
                    tc.tile_pool(name="s1p", bufs=1, space="PSUM") as s1p,
                ):
                    s1_psum = s1p.tile([B, CM], f32)
                    for t in range(n_tiles):
                        xt = xp.tile([2 * K, half * B], mm_dt)
                        # partition p = k + 64*(r%2); host layout
                        # [rp, par, k, ...] makes (par, k) one stride run
                        nc.sync.dma_start(
                            xt[:],
                            _ap(xT[t * half:(t + 1) * half],
                                [(B, 2 * K), (2 * K * B, half), (1, B)]))
                        wt = wp.tile([2 * K, half * CM], mm_dt)
                        nc.sync.dma_start(
                            wt[:],
                            _ap(wT[t * half:(t + 1) * half],
                                [(CM, 2 * K), (2 * K * CM, half), (1, CM)]))
                        ut = up1.tile([B, RT1 * CM], u_dt)
                        for rp in range(half):
                            first = (t == 0 and rp == 0)
                            last = (t == n_tiles - 1 and rp == half - 1)
                            for h in range(2):
                                # packed: u_even + u_odd accumulated into s1
                                nc.tensor.matmul(
                                    s1_psum[:, h * 512:(h + 1) * 512],
                                    xt[:, rp * B:(rp + 1) * B],
                                    wt[:, rp * CM + h * 512:
                                       rp * CM + (h + 1) * 512],
                                    start=first, stop=last,
                                )
                            for par in range(2):
                                j = 2 * rp + par
                                ps = pp.tile([B, CM], f32)
                                for h in range(2):
                                    nc.tensor.matmul(
                                        ps[:, h * 512:(h + 1) * 512],
                                        xt[par * K:(par + 1) * K,
                                           rp * B:(rp + 1) * B],
                                        wt[par * K:(par + 1) * K,
                                           rp * CM + h * 512:
                                           rp * CM + (h + 1) * 512],
                                        start=True, stop=True,
                                    )
                                if j % 2 == 0:
                                    nc.scalar.copy(
                                        ut[:, j * CM:(j + 1) * CM], ps[:])
                                else:
                                    nc.vector.tensor_copy(
                                        ut[:, j * CM:(j + 1) * CM], ps[:])
                        nc.sync.dma_start(
                            u_dram[:, t * RT1 * CM:(t + 1) * RT1 * CM], ut[:])
                    nc.vector.tensor_copy(s_acc[:], s1_psum[:])
                return s_acc

            def emit_phase1_plain():
                s_acc = sm.tile([B, CM], f32, tag="s_acc")
                nc.vector.memset(s_acc[:], 0.0)
                with (
                    tc.tile_pool(name="xp", bufs=3) as xp,
                    tc.tile_pool(name="wp", bufs=3) as wp,
                    tc.tile_pool(name="up1", bufs=3) as up1,
                    tc.tile_pool(name="pp", bufs=4, space="PSUM") as pp,
                ):
                    for t in range(R_LOC // RT1):
                        xt = xp.tile([K, RT1 * B], mm_dt)
                        nc.sync.dma_start(
                            xt[:],
                            _ap(xT[t * RT1 // 2:(t + 1) * RT1 // 2],
                                [(B, K), (K * B, RT1), (1, B)]))
                        wt = wp.tile([K, RT1 * CM], mm_dt)
                        nc.sync.dma_start(
                            wt[:],
                            _ap(wT[t * RT1 // 2:(t + 1) * RT1 // 2],
                                [(CM, K), (K * CM, RT1), (1, CM)]),
                        )
                        ut = up1.tile([B, RT1 * CM], u_dt)
                        for j in range(RT1):
                            ps = pp.tile([B, CM], f32)
                            for h in range(2):
                                nc.tensor.matmul(
                                    ps[:, h * 512:(h + 1) * 512],
                                    xt[:, j * B:(j + 1) * B],
                                    wt[:, j * CM + h * 512:
                                       j * CM + (h + 1) * 512],
                                    start=True, stop=True,
                                )
                            nc.scalar.copy(ut[:, j * CM:(j + 1) * CM], ps[:])
                        # s1 partial: sum over the tile's routes
                        red = sm.tile([B, CM], f32, tag="red")
                        nc.vector.tensor_reduce(
                            red[:], _view(ut[:], [(1, CM), (CM, RT1)]),
                            axis=AX.X, op=op.add)
                        nc.vector.tensor_tensor(s_acc[:], s_acc[:], red[:],
                                                op=op.add)
                        nc.sync.dma_start(
                            u_dram[:, t * RT1 * CM:(t + 1) * RT1 * CM], ut[:])
                return s_acc

            def emit_once():
                # ------------- Phase 1: u production + iteration-1 s ---------
                if S1_ON_PE:
                    s_acc = emit_phase1_packed()
                else:
                    s_acc = emit_phase1_plain()

                allreduce_squash(s_acc, 1.0 / C)

                # ------------- Phase 2: remaining routing iterations ---------
                with (
                    tc.tile_pool(name="up2",
                                 bufs=int(os.environ.get("DC_UPB", "2"))) as up2,
                    tc.tile_pool(name="pq",
                                 bufs=int(os.environ.get("DC_PQB", "2"))) as pq,
                    tc.tile_pool(name="vxp", bufs=1) as vxp,
                    tc.tile_pool(name="cxp", bufs=2) as cxp,
                ):
                    for it in range(2, n_iters + 1):
                        s_acc = sm.tile([B, CM], f32, tag="s_acc")
                        if V_EXP:
                            # v expanded over tile routes (ACT, once/pass)
                            v_exp = vxp.tile([B, RT2 * CM], u_dt, tag="v_exp")
                            nc.scalar.copy(
                                v_exp[:], _view(v_sb[:], [(0, RT2), (1, CM)]))
                        for t in range(R_LOC // RT2):
                            ut = up2.tile([B, RT2 * CM], u_dt)
                            nc.sync.dma_start(
                                ut[:],
                                u_dram[:, t * RT2 * CM:(t + 1) * RT2 * CM])
                            p = pq.tile([B, RT2 * CM], u_dt, tag="pq")
                            if V_EXP:
                                # both contiguous -> DVE 2x mode
                                prod_p.tensor_tensor(p[:], ut[:], v_exp[:],
                                                     op=op.mult)
                            else:
                                prod_p.tensor_tensor(
                                    _view(p[:], [(CM, RT2), (C, M), (1, C)]),
                                    _view(ut[:], [(CM, RT2), (C, M), (1, C)]),
                                    _view(v_u[:], [(0, RT2), (C, M), (1, C)]),
                                    op=op.mult)
                            # dot[b, (r_t, c)] = sum_m p
                            blt = b_log[:, t * RT2 * C:(t + 1) * RT2 * C]
                            if it == 2:
                                nc.vector.tensor_reduce(
                                    blt,
                                    _view(p[:], [(CM, RT2), (1, C), (C, M)]),
                                    axis=AX.X, op=op.add)
                            else:
                                dot = sm.tile([B, RT2 * C], f32, tag="dot")
                                nc.vector.tensor_reduce(
                                    dot[:],
                                    _view(p[:], [(CM, RT2), (1, C), (C, M)]),
                                    axis=AX.X, op=op.add)
                                nc.vector.tensor_tensor(blt, blt, dot[:],
                                                        op=op.add)
                            # softmax over caps (innermost c of blt)
                            mx = sm.tile([B, RT2], f32, tag="mx")
                            nc.vector.tensor_reduce(
                                mx[:], _view(blt, [(C, RT2), (1, C)]),
                                axis=AX.X, op=op.max)
                            e = sm.tile([B, RT2 * C], f32, tag="e")
                            nc.vector.tensor_tensor(
                                _view(e[:], [(C, RT2), (1, C)]),
                                _view(blt, [(C, RT2), (1, C)]),
                                _view(mx[:], [(1, RT2), (0, C)]),
                                op=op.subtract)
                            nc.scalar.activation(
                                e[:], e[:], mybir.ActivationFunctionType.Exp)
                            z = sm.tile([B, RT2], f32, tag="z")
                            nc.vector.tensor_reduce(
                                z[:], _view(e[:], [(C, RT2), (1, C)]),
                                axis=AX.X, op=op.add)
                            nc.vector.reciprocal(z[:], z[:])
                            coef = sm.tile([B, RT2 * C], u_dt, tag="coef")
                            nc.vector.tensor_tensor(
                                _view(coef[:], [(C, RT2), (1, C)]),
                                _view(e[:], [(C, RT2), (1, C)]),
                                _view(z[:], [(1, RT2), (0, C)]),
                                op=op.mult)
                            q = pq.tile([B, RT2 * CM], u_dt, tag="pq")
                            if C_EXP:
                                coef_exp = cxp.tile([B, RT2 * CM], u_dt,
                                                    tag="coef_exp")
                                nc.scalar.copy(
                                    coef_exp[:],
                                    _view(coef[:],
                                          [(C, RT2), (0, M), (1, C)]))
                                prod_q.tensor_tensor(q[:], ut[:],
                                                     coef_exp[:], op=op.mult)
                            elif Q_SWAP:
                                prod_q.tensor_tensor(
                                    _view(q[:], [(CM, RT2), (C, M), (1, C)]),
                                    _view(coef[:],
                                          [(C, RT2), (0, M), (1, C)]),
                                    _view(ut[:], [(CM, RT2), (C, M), (1, C)]),
                                    op=op.mult)
                            else:
                                prod_q.tensor_tensor(
                                    _view(q[:], [(CM, RT2), (C, M), (1, C)]),
                                    _view(ut[:], [(CM, RT2), (C, M), (1, C)]),
                                    _view(coef[:],
                                          [(C, RT2), (0, M), (1, C)]),
                                    op=op.mult)
                            # s partial += sum over r_t of q
                            if t == 0:
                                nc.vector.tensor_reduce(
                                    s_acc[:],
                                    _view(q[:], [(C, M), (1, C), (CM, RT2)]),
                                    axis=AX.X, op=op.add)
                            else:
                                red = sm.tile([B, CM], f32, tag="red")
                                nc.vector.tensor_reduce(
                                    red[:],
                                    _view(q[:], [(C, M), (1, C), (CM, RT2)]),
                                    axis=AX.X, op=op.add)
                                nc.vector.tensor_tensor(s_acc[:], s_acc[:],
                                                        red[:], op=op.add)
                        allreduce_squash(s_acc, 1.0)

            for _ in range(repeat):
                emit_once()

            nc.sync.dma_start(out[:], v_sb[:])

    nc.compile()
    return nc


def kernel(x, route_weights, num_iterations):
    global LAST_RESULT
    from concourse import bass_utils

    n = int(num_iterations)
    assert n >= 1
    x = np.asarray(x, dtype=np.float32)
    w = np.asarray(route_weights, dtype=np.float32)
    assert x.shape == (B, R, K) and w.shape == (R, C, K, M)

    if n not in _compiled:
        _compiled[n] = _build(n)
    nc = _compiled[n]

    in_maps = []
    for c in range(N_CORES):
        sl = slice(c * R_LOC, (c + 1) * R_LOC)
        xT_c = np.ascontiguousarray(
            x[:, sl, :].transpose(1, 2, 0).reshape(R_LOC // 2, 2, K, B))
        wT_c = np.ascontiguousarray(
            w[sl].reshape(R_LOC // 2, 2, C, K, M).transpose(0, 1, 3, 4, 2)
        ).reshape(R_LOC // 2, 2, K, CM)
        in_maps.append({"xT": xT_c, "wT": wT_c})

    res = bass_utils.run_bass_kernel_spmd(
        nc, in_maps, core_ids=list(range(N_CORES)))
    LAST_RESULT = res
    return np.ascontiguousarray(
        res.results[0]["out"].reshape(B, M, C).transpose(0, 2, 1)
    ).astype(np.float32)



# revision 3
# speedup vs baseline: 1.3882x; 1.3882x over previous
"""DigitCaps dynamic-routing kernel for 8 Trainium2 NeuronCores.

Strategy: shard num_route_nodes (R=2048 -> 256/core), fp16 throughout.
  - Phase 0: s1 = sum_r u_r computed as one K=128-packed matmul accumulation
    chain over all local routes (no u materialized). AllReduce -> v1.
  - Phase 1: per 8-route tile: produce u on the PE (fp16 inputs), drain
    PSUM->SBUF on the scalar engine, write u tile to DRAM, and run the
    iteration-2 routing on the tile while it is still in SBUF:
      p = u*v (DVE 2x) -> in-place m-tree (DVE 2x) -> dot -> softmax over
      caps -> q = u*coef (Pool engine on most tiles) -> in-place r-tree
      (DVE 2x) -> s partial.
  - Phase 2 (per extra iteration): stream u tiles back once, same routing.
  s is AllReduced across cores each iteration; squash redundant per core.

All reductions are in-place binary trees of fp16 tensor_tensor adds
(contiguous innermost => DVE 2x mode) instead of tensor_reduce (1x only).
Logits b (= dot2) are stored fp16; softmax uses per-(b,r) max in fp32.
"""

import os
import sys

if "/opt/trn_rl_repo" not in sys.path:
    sys.path.insert(0, "/opt/trn_rl_repo")

import numpy as np

B, R, K, C, M = 128, 2048, 64, 32, 32
CM = C * M
N_CORES = 8
R_LOC = R // N_CORES
RT = int(os.environ.get("DC_RT", "8"))        # routes per tile
PAIRS = RT // 2
NT = R_LOC // RT
POOL_Q = int(os.environ.get("DC_POOLQ", "7"))  # of 8 tiles: q-mult on Pool
SIM_MODE = os.environ.get("DC_SIM", "0") == "1"

_compiled = {}
LAST_RESULT = None


def _view(ap, dims):
    """Free-dim view of an AP: keep its partition dim, replace free dims by
    [step, count] pairs (element steps). step 0 = broadcast."""
    import concourse.bass as bass

    return bass.AP(
        tensor=ap.tensor,
        offset=ap.offset,
        ap=[list(ap.ap[0])] + [[s, c] for s, c in dims],
    )


def _ap(ap, dims):
    """Fully custom AP (all dims given) at the base offset of `ap`."""
    import concourse.bass as bass

    return bass.AP(
        tensor=ap.tensor,
        offset=ap.offset,
        ap=[[s, c] for s, c in dims],
    )


def _squash(nc, pool, s_ap, v_ap):
    """v = s * |s|^2 / ((1 + |s|^2) (sqrt(|s|^2) + 1e-8)), norm over m."""
    import concourse.mybir as mybir

    f32 = mybir.dt.float32
    op = mybir.AluOpType
    sq_full = pool.tile([B, CM], f32, tag="sq_full")
    nc.vector.tensor_tensor(sq_full[:], s_ap, s_ap, op=op.mult)
    sq = pool.tile([B, C], f32, tag="sq")
    nc.vector.tensor_reduce(
        sq[:], _view(sq_full[:], [(1, C), (C, M)]), axis=mybir.AxisListType.X,
        op=op.add)
    rt = pool.tile([B, C], f32, tag="rt")
    nc.scalar.activation(rt[:], sq[:], mybir.ActivationFunctionType.Sqrt)
    nc.vector.tensor_scalar(rt[:], rt[:], 1e-8, None, op0=op.add)
    den = pool.tile([B, C], f32, tag="den")
    nc.vector.tensor_scalar(den[:], sq[:], 1.0, None, op0=op.add)
    nc.vector.tensor_tensor(den[:], den[:], rt[:], op=op.mult)
    fi = pool.tile([B, C], f32, tag="fi")
    nc.vector.reciprocal(fi[:], den[:])
    nc.vector.tensor_tensor(fi[:], fi[:], sq[:], op=op.mult)
    # v = s * f (f broadcast over m)
    nc.vector.tensor_tensor(
        v_ap,
        _view(s_ap, [(C, M), (1, C)]),
        _view(fi[:], [(0, M), (1, C)]),
        op=op.mult,
    )


def _build(n_iters, repeat=1):
    import concourse.mybir as mybir
    import concourse.tile as tile
    from concourse import bacc

    f32 = mybir.dt.float32
    f16 = mybir.dt.float16
    op = mybir.AluOpType
    AX = mybir.AxisListType
    ACT = mybir.ActivationFunctionType

    nc = bacc.Bacc("TRN2", target_bir_lowering=False, debug=False,
                   num_devices=1 if SIM_MODE else N_CORES)
    xT = nc.dram_tensor("xT", [R_LOC // 2, 2, K, B], f16,
                        kind="ExternalInput").ap()
    wT = nc.dram_tensor("wT", [R_LOC // 2, 2, K, CM], f16,
                        kind="ExternalInput").ap()
    out = nc.dram_tensor("out", [B, CM], f32, kind="ExternalOutput").ap()

    with tile.TileContext(nc) as tc:
        with (
            tc.tile_pool(name="sm", bufs=2) as sm,
            tc.tile_pool(name="persist", bufs=1) as persist,
            tc.tile_pool(name="dram", bufs=1, space="DRAM") as dram,
            tc.tile_pool(name="drbounce", bufs=min(2 * (n_iters + 1) * repeat, 8),
                         space="DRAM") as drb,
        ):
            u_dram = dram.tile([B, R_LOC * CM], f16)
            b_log = persist.tile([B, R_LOC * C], f16)   # logits, layout (r, c)
            v_sb = persist.tile([B, CM], f32)
            v_u = persist.tile([B, CM], f16)

            def allreduce_squash(s_acc_tile, scale):
                bin_ = drb.tile([B, CM], f32, tag="bin")
                bout = drb.tile([B, CM], f32, tag="bout")
                nc.sync.dma_start(bin_[:], s_acc_tile[:])
                if SIM_MODE:
                    nc.sync.dma_start(bout[:], bin_[:])
                else:
                    nc.gpsimd.collective_compute(
                        "AllReduce", op.add,
                        replica_groups=[list(range(N_CORES))],
                        ins=[bin_.opt()], outs=[bout.opt()],
                    )
                s_sb = sm.tile([B, CM], f32, tag="s_sb")
                nc.sync.dma_start(s_sb[:], bout[:])
                if scale != 1.0:
                    nc.vector.tensor_scalar(s_sb[:], s_sb[:], scale, None,
                                            op0=op.mult)
                _squash(nc, sm, s_sb[:], v_sb[:])
                nc.vector.tensor_copy(v_u[:], v_sb[:])

            def emit_phase0():
                """s1 = sum_r u_r via one PSUM accumulation chain (K=128)."""
                with (
                    tc.tile_pool(name="x0", bufs=3) as x0,
                    tc.tile_pool(name="w0", bufs=3) as w0,
                    tc.tile_pool(name="s1p", bufs=1, space="PSUM") as s1p,
                ):
                    s1_psum = s1p.tile([B, CM], f32)
                    for t in range(NT):
                        xt = x0.tile([2 * K, PAIRS * B], f16)
                        nc.sync.dma_start(
                            xt[:],
                            _ap(xT[t * PAIRS:(t + 1) * PAIRS],
                                [(B, 2 * K), (2 * K * B, PAIRS), (1, B)]))
                        wt = w0.tile([2 * K, PAIRS * CM], f16)
                        nc.sync.dma_start(
                            wt[:],
                            _ap(wT[t * PAIRS:(t + 1) * PAIRS],
                                [(CM, 2 * K), (2 * K * CM, PAIRS), (1, CM)]))
                        for j in range(PAIRS):
                            first = (t == 0 and j == 0)
                            last = (t == NT - 1 and j == PAIRS - 1)
                            for h in range(2):
                                nc.tensor.matmul(
                                    s1_psum[:, h * 512:(h + 1) * 512],
                                    xt[:, j * B:(j + 1) * B],
                                    wt[:, j * CM + h * 512:j * CM + (h + 1) * 512],
                                    start=first, stop=last,
                                )
                    s_acc = sm.tile([B, CM], f32, tag="s_acc")
                    nc.vector.tensor_copy(s_acc[:], s1_psum[:])
                return s_acc

            def routing_tile(t, ut, it, s_acc, p_pool, q_pool, pending):
                """Iteration-`it` routing work on one 8-route SBUF tile of u.

                Emits: p=u*v, m-tree -> dot, softmax -> coef, q=u*coef,
                and defers the r-tree of the PREVIOUS tile (software skew so
                the DVE r-tree does not stall behind the Pool q-mult)."""
                p = p_pool.tile([B, RT * CM], f16, tag="p")
                nc.vector.tensor_tensor(
                    p[:], ut[:], _view(v_u[:], [(0, RT), (1, CM)]), op=op.mult)
                # in-place m-tree over m (stride C), c contiguous innermost
                s = M // 2
                while s >= 1:
                    dst = _view(p[:], [(CM, RT), (C, s), (1, C)])
                    src = _view(p[:, s * C:], [(CM, RT), (C, s), (1, C)])
                    nc.vector.tensor_tensor(dst, dst, src, op=op.add)
                    s //= 2
                dot_v = _view(p[:], [(CM, RT), (1, C)])  # fp16 [B, (r, c)]
                blt = b_log[:, t * RT * C:(t + 1) * RT * C]
                if it == 2:
                    lg_v = dot_v
                    if n_iters > 2:
                        nc.scalar.copy(_view(blt, [(C, RT), (1, C)]), dot_v)
                else:
                    lg = sm.tile([B, RT * C], f16, tag="lg")
                    lg_v = _view(lg[:], [(C, RT), (1, C)])
                    nc.vector.tensor_tensor(
                        lg_v, _view(blt, [(C, RT), (1, C)]), dot_v, op=op.add)
                    if it < n_iters:
                        nc.scalar.copy(_view(blt, [(C, RT), (1, C)]), lg_v)
                # softmax over caps (innermost c)
                mx = sm.tile([B, RT], f32, tag="mx")
                nc.vector.tensor_reduce(mx[:], lg_v, axis=AX.X, op=op.max)
                e = sm.tile([B, RT * C], f32, tag="e")
                e_v = _view(e[:], [(C, RT), (1, C)])
                nc.vector.tensor_tensor(
                    e_v, lg_v, _view(mx[:], [(1, RT), (0, C)]), op=op.subtract)
                nc.scalar.activation(e[:], e[:], ACT.Exp)
                z = sm.tile([B, RT], f32, tag="z")
                nc.vector.tensor_reduce(z[:], e_v, axis=AX.X, op=op.add)
                nc.vector.reciprocal(z[:], z[:])
                coef = sm.tile([B, RT * C], f16, tag="coef")
                nc.vector.tensor_tensor(
                    _view(coef[:], [(C, RT), (1, C)]), e_v,
                    _view(z[:], [(1, RT), (0, C)]), op=op.mult)
                q = q_pool.tile([B, RT * CM], f16, tag="q")
                q_eng = nc.gpsimd if (t % 8) < POOL_Q else nc.vector
                q_eng.tensor_tensor(
                    _view(q[:], [(CM, RT), (C, M), (1, C)]),
                    _view(ut[:], [(CM, RT), (C, M), (1, C)]),
                    _view(coef[:], [(C, RT), (0, M), (1, C)]),
                    op=op.mult)
                # drain previous tile's q through the r-tree (skewed)
                if pending is not None:
                    rtree_drain(*pending, s_acc)
                return (q, t)

            def rtree_drain(q, t, s_acc):
                s = RT // 2
                while s >= 1:
                    dst = _view(q[:], [(CM, s), (1, CM)])
                    src = _view(q[:, s * CM:], [(CM, s), (1, CM)])
                    nc.vector.tensor_tensor(dst, dst, src, op=op.add)
                    s //= 2
                if t == 0:
                    nc.vector.tensor_copy(s_acc[:], q[:, :CM])
                else:
                    nc.vector.tensor_tensor(s_acc[:], s_acc[:], q[:, :CM],
                                            op=op.add)

            def emit_phase1(s_acc):
                """u production fused with iteration-2 routing."""
                pending = None
                with (
                    tc.tile_pool(name="x1", bufs=3) as x1,
                    tc.tile_pool(name="w1", bufs=3) as w1,
                    tc.tile_pool(name="up", bufs=3) as up,
                    tc.tile_pool(name="pp", bufs=4, space="PSUM") as pp,
                    tc.tile_pool(name="ppool", bufs=2) as p_pool,
                    tc.tile_pool(name="qpool", bufs=2) as q_pool,
                ):
                    for t in range(NT):
                        xt = x1.tile([2 * K, PAIRS * B], f16)
                        nc.sync.dma_start(
                            xt[:],
                            _ap(xT[t * PAIRS:(t + 1) * PAIRS],
                                [(B, 2 * K), (2 * K * B, PAIRS), (1, B)]))
                        wt = w1.tile([2 * K, PAIRS * CM], f16)
                        nc.sync.dma_start(
                            wt[:],
                            _ap(wT[t * PAIRS:(t + 1) * PAIRS],
                                [(CM, 2 * K), (2 * K * CM, PAIRS), (1, CM)]))
                        ut = up.tile([B, RT * CM], f16)
                        for j in range(PAIRS):
                            for par in range(2):
                                r_idx = 2 * j + par
                                ps = pp.tile([B, CM], f32)
                                for h in range(2):
                                    nc.tensor.matmul(
                                        ps[:, h * 512:(h + 1) * 512],
                                        xt[par * K:(par + 1) * K,
                                           j * B:(j + 1) * B],
                                        wt[par * K:(par + 1) * K,
                                           j * CM + h * 512:
                                           j * CM + (h + 1) * 512],
                                        start=True, stop=True,
                                    )
                                nc.scalar.copy(
                                    ut[:, r_idx * CM:(r_idx + 1) * CM], ps[:])
                        nc.sync.dma_start(
                            u_dram[:, t * RT * CM:(t + 1) * RT * CM], ut[:])
                        pending = routing_tile(t, ut, 2, s_acc,
                                               p_pool, q_pool, pending)
                    rtree_drain(*pending, s_acc)

            def emit_phase2(it, s_acc):
                """One streaming routing pass over staged u."""
                pending = None
                with (
                    tc.tile_pool(name="up2", bufs=3) as up,
                    tc.tile_pool(name="ppool2", bufs=2) as p_pool,
                    tc.tile_pool(name="qpool2", bufs=3) as q_pool,
                ):
                    for t in range(NT):
                        ut = up.tile([B, RT * CM], f16)
                        nc.sync.dma_start(
                            ut[:], u_dram[:, t * RT * CM:(t + 1) * RT * CM])
                        pending = routing_tile(t, ut, it, s_acc,
                                               p_pool, q_pool, pending)
                    rtree_drain(*pending, s_acc)

            def emit_once():
                s_acc = emit_phase0()
                allreduce_squash(s_acc, 1.0 / C)
                if n_iters >= 2:
                    s_acc2 = sm.tile([B, CM], f32, tag="s_acc")
                    emit_phase1(s_acc2)
                    allreduce_squash(s_acc2, 1.0)
                for it in range(3, n_iters + 1):
                    s_acc3 = sm.tile([B, CM], f32, tag="s_acc")
                    emit_phase2(it, s_acc3)
                    allreduce_squash(s_acc3, 1.0)

            for _ in range(repeat):
                emit_once()

            nc.sync.dma_start(out[:], v_sb[:])

    nc.compile()
    return nc


def make_in_maps(x, w):
    """Host-side shard + layout prep: fp16, route pairs packed on 128
    partitions, weight columns (m, c) with c innermost."""
    in_maps = []
    for c in range(N_CORES):
        sl = slice(c * R_LOC, (c + 1) * R_LOC)
        xT_c = np.ascontiguousarray(
            x[:, sl, :].transpose(1, 2, 0).reshape(R_LOC // 2, 2, K, B)
        ).astype(np.float16)
        wT_c = np.ascontiguousarray(
            w[sl].reshape(R_LOC // 2, 2, C, K, M).transpose(0, 1, 3, 4, 2)
        ).reshape(R_LOC // 2, 2, K, CM).astype(np.float16)
        in_maps.append({"xT": xT_c, "wT": wT_c})
    return in_maps


def kernel(x, route_weights, num_iterations):
    global LAST_RESULT
    from concourse import bass_utils

    n = int(num_iterations)
    assert n >= 1
    x = np.asarray(x, dtype=np.float32)
    w = np.asarray(route_weights, dtype=np.float32)
    assert x.shape == (B, R, K) and w.shape == (R, C, K, M)

    if n not in _compiled:
        _compiled[n] = _build(n)
    nc = _compiled[n]

    in_maps = make_in_maps(x, w)
    res = bass_utils.run_bass_kernel_spmd(
        nc, in_maps, core_ids=list(range(N_CORES)))
    LAST_RESULT = res
    return np.ascontiguousarray(
        res.results[0]["out"].reshape(B, M, C).transpose(0, 2, 1)
    ).astype(np.float32)


# revision 25
# speedup vs baseline: 3.2852x; 2.3666x over previous
"""DigitCaps dynamic-routing kernel for 8 Trainium2 NeuronCores.

Strategy: shard num_route_nodes (R=2048 -> 256/core), fp16 throughout.
  - Phase 0: s1 = sum_r u_r as one K=128-packed matmul accumulation chain
    over all local routes (u never materialized). AllReduce -> v1.
  - Phase 1: per 8-route tile: produce u on the PE (fp16 inputs), drain
    PSUM->SBUF on the scalar engine, write the u tile to DRAM, and run the
    iteration-2 routing on the tile while it is still in SBUF.
  - Phase 2 (per extra iteration): stream u tiles back once, same routing.

Routing per tile -- both contractions ride the tensor engine:
  p = u*v                              (DVE, fp16 2x mode)
  dot: identity-matmul transposes of p 128-col chunks PSUM-accumulate the
       m-subgroups per route (dotp[(m%4)*32+c, b]), then one matmul against
       a block-ones matrix finishes sum_m and lands dot[b, (r,c)] directly.
  softmax over caps                    (DVE + ACT exp, per-(b,r) max)
  q = u*coef                           (split DVE/Pool)
  s: identity-matmul transposes of q PSUM-accumulate over routes AND tiles
     into s_T[cm%128, (chunk, b)] -- the whole r-contraction costs zero
     vector cycles.
s_T is AllReduced (fp16) across cores; squash runs in the transposed layout
and v returns to [b, cm] via more identity matmuls.  PE emission is
software-pipelined (1-2 tile skew) so no engine head-of-line blocks another.
"""

import os
import sys

if "/opt/trn_rl_repo" not in sys.path:
    sys.path.insert(0, "/opt/trn_rl_repo")

import numpy as np

B, R, K, C, M = 128, 2048, 64, 32, 32
CM = C * M
N_CORES = 8
R_LOC = R // N_CORES
RT = 8                                         # routes per tile
PAIRS = RT // 2
NT = R_LOC // RT
NCH = CM // 128                                # 128-col chunks per route (8)
POOL_Q = int(os.environ.get("DC_POOLQ", "4"))  # of 8 tiles: q-mult on Pool
AR16 = os.environ.get("DC_AR16", "0") == "1"   # fp16 collectives
SIM_MODE = os.environ.get("DC_SIM", "0") == "1"

_compiled = {}
LAST_RESULT = None


def _view(ap, dims):
    """Free-dim view of an AP: keep its partition dim, replace free dims by
    [step, count] pairs (element steps). step 0 = broadcast."""
    import concourse.bass as bass

    return bass.AP(
        tensor=ap.tensor,
        offset=ap.offset,
        ap=[list(ap.ap[0])] + [[s, c] for s, c in dims],
    )


def _ap(ap, dims):
    """Fully custom AP (all dims given) at the base offset of `ap`."""
    import concourse.bass as bass

    return bass.AP(
        tensor=ap.tensor,
        offset=ap.offset,
        ap=[[s, c] for s, c in dims],
    )


def _build(n_iters, repeat=1):
    import concourse.mybir as mybir
    import concourse.tile as tile
    from concourse import bacc

    f32 = mybir.dt.float32
    f16 = mybir.dt.float16
    ar_dt = f16 if AR16 else f32
    op = mybir.AluOpType
    AX = mybir.AxisListType
    ACT = mybir.ActivationFunctionType

    nc = bacc.Bacc("TRN2", target_bir_lowering=False, debug=False,
                   num_devices=1 if SIM_MODE else N_CORES)
    xT = nc.dram_tensor("xT", [R_LOC // 2, 2, K, B], f16,
                        kind="ExternalInput").ap()
    wT = nc.dram_tensor("wT", [R_LOC // 2, 2, K, CM], f16,
                        kind="ExternalInput").ap()
    idin = nc.dram_tensor("idin", [128, 128], f16, kind="ExternalInput").ap()
    idin32 = nc.dram_tensor("idin32", [128, 128], f32,
                            kind="ExternalInput").ap()
    obin = nc.dram_tensor("obin", [128, C], f32, kind="ExternalInput").ap()
    out = nc.dram_tensor("out", [B, CM], f32, kind="ExternalOutput").ap()
    DEBUG = os.environ.get("DC_DEBUG", "0") == "1"
    if DEBUG:
        dbg = nc.dram_tensor("dbg", [B, R_LOC * C], mybir.dt.float16,
                             kind="ExternalOutput").ap()

    with tile.TileContext(nc) as tc:
        with (
            tc.tile_pool(name="sm", bufs=2) as sm,
            tc.tile_pool(name="persist", bufs=1) as persist,
            tc.tile_pool(name="dram", bufs=1, space="DRAM") as dram,
            tc.tile_pool(name="drbounce", bufs=min(2 * (n_iters + 1) * repeat, 8),
                         space="DRAM") as drb,
        ):
            u_dram = dram.tile([B, R_LOC * CM], f16)
            b_log = persist.tile([B, R_LOC * C], f16)   # logits, layout (r, c)
            v_sb = persist.tile([B, CM], f32)
            v_u = persist.tile([B, CM], f16)
            ident = persist.tile([128, 128], f16)
            ident32 = persist.tile([128, 128], f32)
            oblk = persist.tile([128, C], f32)          # [p,c]=1 iff p%32==c
            nc.sync.dma_start(ident[:], idin)
            nc.sync.dma_start(ident32[:], idin32)
            nc.sync.dma_start(oblk[:], obin)

            def squash_T(s_sbT, scale):
                """squash on s_T[cm%128, (chunk, b)]; writes v_u/v_sb[b, cm].

                norm over m: chunk ci holds m in [4ci,4ci+4), partition
                p = (m%4)*32 + c."""
                with tc.tile_pool(name="vps", bufs=1, space="PSUM") as vps:
                    if scale != 1.0:
                        nc.vector.tensor_scalar(s_sbT[:], s_sbT[:], scale,
                                                None, op0=op.mult)
                    sq2 = sm.tile([128, NCH * B], f32, tag="sq2")
                    nc.vector.tensor_tensor(sq2[:], s_sbT[:], s_sbT[:],
                                            op=op.mult)
                    sqp = sm.tile([128, B], f32, tag="sqp")
                    nc.vector.tensor_reduce(
                        sqp[:], _view(sq2[:], [(1, B), (B, NCH)]), axis=AX.X,
                        op=op.add)
                    spt = vps.tile([128, 128], f32, tag="spt")
                    nc.tensor.matmul(spt[:], sqp[:], ident32[:],
                                     start=True, stop=True)
                    sq = sm.tile([B, C], f32, tag="sq")
                    nc.vector.tensor_reduce(
                        sq[:], _view(spt[:], [(1, C), (C, 4)]), axis=AX.X,
                        op=op.add)
                    rt = sm.tile([B, C], f32, tag="rt")
                    nc.scalar.activation(rt[:], sq[:], ACT.Sqrt)
                    nc.vector.tensor_scalar(rt[:], rt[:], 1e-8, None,
                                            op0=op.add)
                    den = sm.tile([B, C], f32, tag="den")
                    nc.vector.tensor_scalar(den[:], sq[:], 1.0, None,
                                            op0=op.add)
                    nc.vector.tensor_tensor(den[:], den[:], rt[:], op=op.mult)
                    fi = sm.tile([B, C], f32, tag="fi")
                    nc.vector.reciprocal(fi[:], den[:])
                    nc.vector.tensor_tensor(fi[:], fi[:], sq[:], op=op.mult)
                    # transpose s back to [b, cm] and apply fi
                    if s_sbT.dtype != f16:
                        s16 = sm.tile([128, NCH * B], f16, tag="s16")
                        nc.vector.tensor_copy(s16[:], s_sbT[:])
                        s16v = s16
                    else:
                        s16v = s_sbT
                    vp = vps.tile([B, CM], f32, tag="vp")
                    for ci in range(NCH):
                        nc.tensor.matmul(
                            vp[:, ci * 128:(ci + 1) * 128],
                            s16v[:, ci * B:(ci + 1) * B], ident[:],
                            start=(ci % 4 == 0), stop=(ci % 4 == 3))
                    fi_b = _view(fi[:], [(0, M), (1, C)])
                    vp_v = _view(vp[:], [(C, M), (1, C)])
                    nc.vector.tensor_tensor(
                        _view(v_u[:], [(C, M), (1, C)]), vp_v, fi_b,
                        op=op.mult)
                    nc.vector.tensor_tensor(
                        _view(v_sb[:], [(C, M), (1, C)]), vp_v, fi_b,
                        op=op.mult)

            def allreduce_squash_T(sT_psum, scale):
                sT_sb = sm.tile([128, NCH * B], ar_dt, tag="sT_sb")
                nc.vector.tensor_copy(sT_sb[:], sT_psum[:])
                bin_ = drb.tile([128, NCH * B], ar_dt, tag="bin")
                bout = drb.tile([128, NCH * B], ar_dt, tag="bout")
                nc.sync.dma_start(bin_[:], sT_sb[:])
                if SIM_MODE:
                    nc.sync.dma_start(bout[:], bin_[:])
                else:
                    nc.gpsimd.collective_compute(
                        "AllReduce", op.add,
                        replica_groups=[list(range(N_CORES))],
                        ins=[bin_.opt()], outs=[bout.opt()],
                    )
                s2 = sm.tile([128, NCH * B], ar_dt, tag="s2")
                nc.sync.dma_start(s2[:], bout[:])
                squash_T(s2, scale)

            def emit_phase0(sT_pool):
                """s1 = sum_r u_r via one PSUM chain; transpose into s_T."""
                with (
                    tc.tile_pool(name="x0", bufs=3) as x0,
                    tc.tile_pool(name="w0", bufs=3) as w0,
                    tc.tile_pool(name="s1p", bufs=1, space="PSUM") as s1p,
                ):
                    s1_psum = s1p.tile([B, CM], f32)
                    for t in range(NT):
                        xt = x0.tile([2 * K, PAIRS * B], f16)
                        nc.sync.dma_start(
                            xt[:],
                            _ap(xT[t * PAIRS:(t + 1) * PAIRS],
                                [(B, 2 * K), (2 * K * B, PAIRS), (1, B)]))
                        wt = w0.tile([2 * K, PAIRS * CM], f16)
                        nc.sync.dma_start(
                            wt[:],
                            _ap(wT[t * PAIRS:(t + 1) * PAIRS],
                                [(CM, 2 * K), (2 * K * CM, PAIRS), (1, CM)]))
                        for j in range(PAIRS):
                            first = (t == 0 and j == 0)
                            last = (t == NT - 1 and j == PAIRS - 1)
                            for h in range(2):
                                nc.tensor.matmul(
                                    s1_psum[:, h * 512:(h + 1) * 512],
                                    xt[:, j * B:(j + 1) * B],
                                    wt[:, j * CM + h * 512:j * CM + (h + 1) * 512],
                                    start=first, stop=last,
                                )
                    s1_16 = sm.tile([B, CM], f16, tag="s1_16")
                    nc.scalar.copy(s1_16[:], s1_psum[:])
                    sT = sT_pool.tile([128, NCH * B], f32, tag="sT")
                    for ci in range(NCH):
                        nc.tensor.matmul(
                            sT[:, ci * B:(ci + 1) * B],
                            s1_16[:, ci * 128:(ci + 1) * 128], ident[:],
                            start=(ci % 4 == 0), stop=(ci % 4 == 3))
                return sT

            def dve_tile(t, ut, it, p_pool, q_pool, dp_pool, db_pool):
                """Vector-side routing for one tile; PE parts emitted
                separately (skewed). Returns (p, dotp, dotp32, dot_b, q)."""
                p = p_pool.tile([B, RT * CM], f16, tag="p")
                nc.vector.tensor_tensor(
                    p[:], ut[:], _view(v_u[:], [(0, RT), (1, CM)]), op=op.mult)
                dotp = dp_pool.tile([128, RT * B], f32)      # PSUM, per-r regions
                dotp32 = sm.tile([128, RT * B], f32, tag="dotp32")
                dot_b = db_pool.tile([B, RT * C], f32)       # PSUM
                return p, dotp, dotp32, dot_b

            def pe_dot(p, dotp, dotp32, dot_b):
                """sum_m on the PE: transpose-accumulate p chunks per route,
                drain via ACT, finish with the block-ones matmul."""
                for g in range(RT // 4):
                    for r in range(g * 4, g * 4 + 4):
                        for ci in range(NCH):
                            nc.tensor.matmul(
                                dotp[:, r * B:(r + 1) * B],
                                p[:, r * CM + ci * 128:r * CM + (ci + 1) * 128],
                                ident[:],
                                start=(r % 4 == 0 and ci == 0),
                                stop=(r % 4 == 3 and ci == NCH - 1))
                    nc.scalar.copy(dotp32[:, g * 4 * B:(g + 1) * 4 * B],
                                   dotp[:, g * 4 * B:(g + 1) * 4 * B])
                for r in range(RT):
                    nc.tensor.matmul(
                        dot_b[:, r * C:(r + 1) * C],
                        dotp32[:, r * B:(r + 1) * B], oblk[:],
                        start=(r == 0), stop=(r == RT - 1))

            def softmax_q(t, ut, it, dot_b, q_pool):
                """softmax over caps from dot_b (PSUM) + q-mult."""
                blt = b_log[:, t * RT * C:(t + 1) * RT * C]
                dot_v = _view(dot_b[:], [(C, RT), (1, C)])
                if it == 2:
                    lg_v = dot_v
                    if n_iters > 2:
                        nc.scalar.copy(_view(blt, [(C, RT), (1, C)]), dot_v)
                else:
                    lg = sm.tile([B, RT * C], f16, tag="lg")
                    lg_v = _view(lg[:], [(C, RT), (1, C)])
                    nc.vector.tensor_tensor(
                        lg_v, _view(blt, [(C, RT), (1, C)]), dot_v, op=op.add)
                    if it < n_iters:
                        nc.scalar.copy(_view(blt, [(C, RT), (1, C)]), lg_v)
                mx = sm.tile([B, RT], f32, tag="mx")
                nc.vector.tensor_reduce(mx[:], lg_v, axis=AX.X, op=op.max)
                e = sm.tile([B, RT * C], f32, tag="e")
                e_v = _view(e[:], [(C, RT), (1, C)])
                nc.vector.tensor_tensor(
                    e_v, lg_v, _view(mx[:], [(1, RT), (0, C)]), op=op.subtract)
                nc.scalar.activation(e[:], e[:], ACT.Exp)
                z = sm.tile([B, RT], f32, tag="z")
                nc.vector.tensor_reduce(z[:], e_v, axis=AX.X, op=op.add)
                nc.vector.reciprocal(z[:], z[:])
                coef = sm.tile([B, RT * C], f16, tag="coef")
                nc.vector.tensor_tensor(
                    _view(coef[:], [(C, RT), (1, C)]), e_v,
                    _view(z[:], [(1, RT), (0, C)]), op=op.mult)
                if DEBUG and it == n_iters:
                    nc.scalar.copy(_view(blt, [(C, RT), (1, C)]),
                                   _view(coef[:], [(C, RT), (1, C)]))
                q = q_pool.tile([B, RT * CM], f16, tag="q")
                q_eng = nc.gpsimd if (t % 8) < POOL_Q else nc.vector
                q_eng.tensor_tensor(
                    _view(q[:], [(CM, RT), (C, M), (1, C)]),
                    _view(ut[:], [(CM, RT), (C, M), (1, C)]),
                    _view(coef[:], [(C, RT), (0, M), (1, C)]),
                    op=op.mult)
                return q

            def pe_rsum(q, t, sT):
                """r-sum on the PE: transpose-accumulate q chunks into s_T."""
                for r in range(RT):
                    for ci in range(NCH):
                        # start/stop once per 2KB PSUM bank: start lazily
                        # marks the WHOLE bank pending-zero, so each bank
                        # must see exactly one start (its first write)
                        nc.tensor.matmul(
                            sT[:, ci * B:(ci + 1) * B],
                            q[:, r * CM + ci * 128:r * CM + (ci + 1) * 128],
                            ident[:],
                            start=(t == 0 and r == 0 and ci % 4 == 0),
                            stop=(t == NT - 1 and r == RT - 1 and ci % 4 == 3))

            def emit_phase1(sT):
                """u production fused with iteration-2 routing, software-
                pipelined: PE order is u-mms(t), dot-path(t-1), q-rsum(t-2)."""
                stage = {}
                with (
                    tc.tile_pool(name="x1", bufs=3) as x1,
                    tc.tile_pool(name="w1", bufs=2) as w1,
                    tc.tile_pool(name="up", bufs=3) as up,
                    tc.tile_pool(name="pp", bufs=3, space="PSUM") as pp,
                    tc.tile_pool(name="dpp", bufs=1, space="PSUM") as dpp,
                    tc.tile_pool(name="dbp", bufs=1, space="PSUM") as dbp,
                    tc.tile_pool(name="ppool", bufs=2) as p_pool,
                    tc.tile_pool(name="qpool", bufs=2) as q_pool,
                ):
                    for t in range(NT + 2):
                        if t < NT:
                            xt = x1.tile([2 * K, PAIRS * B], f16)
                            nc.sync.dma_start(
                                xt[:],
                                _ap(xT[t * PAIRS:(t + 1) * PAIRS],
                                    [(B, 2 * K), (2 * K * B, PAIRS), (1, B)]))
                            wt = w1.tile([2 * K, PAIRS * CM], f16)
                            nc.sync.dma_start(
                                wt[:],
                                _ap(wT[t * PAIRS:(t + 1) * PAIRS],
                                    [(CM, 2 * K), (2 * K * CM, PAIRS),
                                     (1, CM)]))
                            ut = up.tile([B, RT * CM], f16)
                            for r_idx in range(RT):
                                j, par = divmod(r_idx, 2)
                                for h in range(2):
                                    ps = pp.tile([B, 512], f32)
                                    nc.tensor.matmul(
                                        ps[:],
                                        xt[par * K:(par + 1) * K,
                                           j * B:(j + 1) * B],
                                        wt[par * K:(par + 1) * K,
                                           j * CM + h * 512:
                                           j * CM + (h + 1) * 512],
                                        start=True, stop=True,
                                    )
                                    nc.scalar.copy(
                                        ut[:, r_idx * CM + h * 512:
                                           r_idx * CM + (h + 1) * 512], ps[:])
                            nc.sync.dma_start(
                                u_dram[:, t * RT * CM:(t + 1) * RT * CM],
                                ut[:])
                            stage[t] = [ut, None, None]
                        if t - 1 >= 0 and t - 1 < NT:
                            ut1 = stage[t - 1][0]
                            p, dotp, dotp32, dot_b = dve_tile(
                                t - 1, ut1, 2, p_pool, q_pool, dpp, dbp)
                            pe_dot(p, dotp, dotp32, dot_b)
                            q = softmax_q(t - 1, ut1, 2, dot_b, q_pool)
                            stage[t - 1][1] = q
                        if t - 2 >= 0:
                            pe_rsum(stage[t - 2][1], t - 2, sT)
                            del stage[t - 2]

            def emit_phase2(it, sT):
                """One streaming routing pass over staged u (1-tile skew on
                the PE q-rsum)."""
                stage = {}
                with (
                    tc.tile_pool(name="up2", bufs=3) as up,
                    tc.tile_pool(name="dpp2", bufs=1, space="PSUM") as dpp,
                    tc.tile_pool(name="dbp2", bufs=1, space="PSUM") as dbp,
                    tc.tile_pool(name="ppool2", bufs=2) as p_pool,
                    tc.tile_pool(name="qpool2", bufs=2) as q_pool,
                ):
                    for t in range(NT + 1):
                        if t < NT:
                            ut = up.tile([B, RT * CM], f16)
                            nc.sync.dma_start(
                                ut[:],
                                u_dram[:, t * RT * CM:(t + 1) * RT * CM])
                            p, dotp, dotp32, dot_b = dve_tile(
                                t, ut, it, p_pool, q_pool, dpp, dbp)
                            pe_dot(p, dotp, dotp32, dot_b)
                            q = softmax_q(t, ut, it, dot_b, q_pool)
                            stage[t] = q
                        if t - 1 >= 0:
                            pe_rsum(stage[t - 1], t - 1, sT)
                            del stage[t - 1]

            def emit_once():
                with tc.tile_pool(name="sTp0", bufs=1, space="PSUM") as sTp:
                    sT = emit_phase0(sTp)
                    allreduce_squash_T(sT, 1.0 / C)
                if n_iters >= 2:
                    with tc.tile_pool(name="sTp1", bufs=1, space="PSUM") as sTp:
                        sT = sTp.tile([128, NCH * B], f32, tag="sT")
                        emit_phase1(sT)
                        allreduce_squash_T(sT, 1.0)
                for it in range(3, n_iters + 1):
                    with tc.tile_pool(name="sTp2", bufs=1, space="PSUM") as sTp:
                        sT = sTp.tile([128, NCH * B], f32, tag="sT")
                        emit_phase2(it, sT)
                        allreduce_squash_T(sT, 1.0)

            for _ in range(repeat):
                emit_once()

            nc.sync.dma_start(out[:], v_sb[:])
            if DEBUG:
                nc.sync.dma_start(dbg, b_log[:])

    nc.compile()
    return nc


def make_in_maps(x, w):
    """Host-side shard + layout prep: fp16, route pairs packed on 128
    partitions, weight columns (m, c) with c innermost."""
    ident = np.eye(128, dtype=np.float16)
    ident32 = np.eye(128, dtype=np.float32)
    oblk = np.zeros((128, C), dtype=np.float32)
    oblk[np.arange(128), np.arange(128) % C] = 1.0
    in_maps = []
    for c in range(N_CORES):
        sl = slice(c * R_LOC, (c + 1) * R_LOC)
        xT_c = np.ascontiguousarray(
            x[:, sl, :].transpose(1, 2, 0).reshape(R_LOC // 2, 2, K, B)
        ).astype(np.float16)
        wT_c = np.ascontiguousarray(
            w[sl].reshape(R_LOC // 2, 2, C, K, M).transpose(0, 1, 3, 4, 2)
        ).reshape(R_LOC // 2, 2, K, CM).astype(np.float16)
        in_maps.append({"xT": xT_c, "wT": wT_c, "idin": ident,
                        "idin32": ident32, "obin": oblk})
    return in_maps


def kernel(x, route_weights, num_iterations):
    global LAST_RESULT
    from concourse import bass_utils

    n = int(num_iterations)
    assert n >= 1
    x = np.asarray(x, dtype=np.float32)
    w = np.asarray(route_weights, dtype=np.float32)
    assert x.shape == (B, R, K) and w.shape == (R, C, K, M)

    if n not in _compiled:
        _compiled[n] = _build(n)
    nc = _compiled[n]

    in_maps = make_in_maps(x, w)
    res = bass_utils.run_bass_kernel_spmd(
        nc, in_maps, core_ids=list(range(N_CORES)))
    LAST_RESULT = res
    return np.ascontiguousarray(
        res.results[0]["out"].reshape(B, M, C).transpose(0, 2, 1)
    ).astype(np.float32)
